# revision 5
# baseline (speedup 1.0000x reference)
"""Binarized linear: out = sign(x+eps) @ sign(w+eps).T on 8 trn2 cores.

Sharding: 4x2 grid. Core c=(r,s): rows x[r*2048:(r+1)*2048], rows w[s*2048:(s+1)*2048].
Each core computes a [2048, 2048] output block; host concatenates. No collectives.

Per-core kernel:
  - binarize x-shard to fp8e4m3 as +/-0.5 (DVE: (x>=0)-0.5), w-shard as +/-1 (ACT Sign)
  - transpose both to [K-on-partition] layout via plain fp8 matmul against identity
    (PE, exact, counts as PE-busy so HAM stays warm)
  - fp8 DoubleRow matmuls accumulate K=256 per instruction into fp32 PSUM
  - out = 2 * psum (exact: sums of +/-0.5 are multiples of 0.5; integers < 2^24)
"""

import numpy as np

P = 128
GRID_I, GRID_J = 4, 2
N_CORES = 8
FULL_M, FULL_N, FULL_K = 8192, 4096, 4096
M_SH, N_SH = FULL_M // GRID_I, FULL_N // GRID_J  # 2048, 2048

_PROGRAM_CACHE = {}


def build_program(m_sh=M_SH, n_sh=N_SH, k=FULL_K, use_dr=True):
    """Build (and cache) the per-core Bass program. Same SPMD program on all cores."""
    key = (m_sh, n_sh, k, use_dr)
    if key in _PROGRAM_CACHE:
        return _PROGRAM_CACHE[key]

    from contextlib import ExitStack

    import concourse.bass as bass
    import concourse.mybir as mybir
    from concourse import bacc, tile
    from concourse.masks import make_identity

    f32 = mybir.dt.float32
    fp8 = mybir.dt.float8e4

    KT = k // P          # number of 128-wide k tiles
    IB = m_sh // P       # i blocks (x rows / 128)
    JBLK = 512           # j chunk width (matmul free dim)
    JC = n_sh // JBLK    # j chunks
    JB_PER_JC = JBLK // P
    KG = 4               # k-tiles per transpose-evict group
    assert KT % KG == 0 and KT % 2 == 0

    nc = bacc.Bacc("TRN2", target_bir_lowering=False, debug=False)
    xs = nc.dram_tensor("xs", [m_sh, k], f32, kind="ExternalInput").ap()
    ws = nc.dram_tensor("ws", [n_sh, k], f32, kind="ExternalInput").ap()
    out = nc.dram_tensor("out", [m_sh, n_sh], f32, kind="ExternalOutput").ap()

    with tile.TileContext(nc) as tc, ExitStack() as ctx:
        const_pool = ctx.enter_context(tc.tile_pool(name="const", bufs=1))
        stage_pool = ctx.enter_context(tc.tile_pool(name="stage", bufs=2))
        b8_pool = ctx.enter_context(tc.tile_pool(name="b8", bufs=2))
        xbt_pool = ctx.enter_context(tc.tile_pool(name="xbt", bufs=1))
        wbt_pool = ctx.enter_context(tc.tile_pool(name="wbt", bufs=1))
        out_pool = ctx.enter_context(tc.tile_pool(name="outp", bufs=4))
        psum_t = ctx.enter_context(tc.tile_pool(name="pst", bufs=2, space="PSUM"))
        psum_mm = ctx.enter_context(tc.tile_pool(name="psmm", bufs=2, space="PSUM"))

        ident = const_pool.tile([P, P], fp8, tag="ident")
        make_identity(nc, ident)
        sign_bias = const_pool.tile([P, 1], f32, tag="sbias")
        nc.any.memset(sign_bias[:], 1e-20)

        # Resident transposed binarized operands, K on partitions:
        #   xbT[ib][kp, kt, i] = bin(xs[ib*128 + i, kt*128 + kp])   (+/-0.5)
        #   wbT[jc][kp, kt, j] = bin(ws[jc*512 + j, kt*128 + kp])   (+/-1)
        xbT = [
            xbt_pool.tile([P, KT, P], fp8, tag=f"xbt{ib}", name=f"xbt{ib}")
            for ib in range(IB)
        ]
        wbT = [
            wbt_pool.tile([P, KT, JBLK], fp8, tag=f"wbt{jc}", name=f"wbt{jc}")
            for jc in range(JC)
        ]

        def prep_block(src_rows, b8_tag, binarize, dest, dest_col0):
            """Load 128 rows x k f32, binarize to fp8, PE-transpose into dest."""
            stg = stage_pool.tile([P, k], f32, tag="stg")
            nc.sync.dma_start(stg[:], src_rows)
            b8 = b8_pool.tile([P, k], fp8, tag=b8_tag)
            binarize(b8, stg)
            for kg in range(KT // KG):
                pt = psum_t.tile([P, KG, P], f32, tag="pt")
                for t in range(KG):
                    kt = kg * KG + t
                    nc.tensor.matmul(
                        pt[:, t, :],
                        lhsT=b8[:, kt * P:(kt + 1) * P],
                        rhs=ident[:],
                        start=True,
                        stop=True,
                    )
                nc.any.tensor_copy(
                    dest[:, kg * KG:(kg + 1) * KG, dest_col0:dest_col0 + P],
                    pt[:],
                )

        def bin_x(b8, stg):
            # (x >= 0) -> {1,0}; minus 0.5 -> +/-0.5. Matches sign(x+1e-20) up to
            # measure-zero region (-1e-20, 0) which float32 randn never hits.
            nc.vector.tensor_scalar(
                b8[:], stg[:], 0.0, 0.5,
                mybir.AluOpType.is_ge, mybir.AluOpType.subtract,
            )

        def bin_w(b8, stg):
            nc.scalar.sign(b8[:], stg[:], bias=sign_bias[:])  # sign(w+1e-20) -> +/-1

        def mm_block(ib, jc):
            ps = psum_mm.tile([P, JBLK], f32, tag="ps")
            if use_dr:
                for kp in range(KT // 2):
                    nc.tensor.matmul(
                        ps[:],
                        lhsT=xbT[ib][:, 2 * kp:2 * kp + 2, :],
                        rhs=wbT[jc][:, 2 * kp:2 * kp + 2, :],
                        start=(kp == 0),
                        stop=(kp == KT // 2 - 1),
                        perf_mode=mybir.MatmulPerfMode.DoubleRow,
                    )
            else:
                for kt in range(KT):
                    nc.tensor.matmul(
                        ps[:],
                        lhsT=xbT[ib][:, kt, :],
                        rhs=wbT[jc][:, kt, :],
                        start=(kt == 0),
                        stop=(kt == KT - 1),
                    )
            ob = out_pool.tile([P, JBLK], f32, tag="ob")
            # products are +/-0.5 (x) * +/-1 (w) = +/-0.5 -> scale by 2
            nc.any.tensor_scalar_mul(ob[:], ps[:], 2.0)
            nc.sync.dma_start(
                out[ib * P:(ib + 1) * P, jc * JBLK:(jc + 1) * JBLK], ob[:]
            )

        # Phase A: prep all of w (DMA-bound; PE does the transposes)
        for jb in range(n_sh // P):
            jc, sub = divmod(jb, JB_PER_JC)
            prep_block(ws[jb * P:(jb + 1) * P, :], "wb8", bin_w, wbT[jc], sub * P)

        # Phase B: prep x interleaved with first j-chunk of matmuls
        for ib in range(IB):
            prep_block(xs[ib * P:(ib + 1) * P, :], "xb8", bin_x, xbT[ib], 0)
            mm_block(ib, 0)

        # Phase C: remaining j-chunks (PE-bound, pure DoubleRow matmuls)
        for jc in range(1, JC):
            for ib in range(IB):
                mm_block(ib, jc)

    nc.compile()
    _PROGRAM_CACHE[key] = nc
    return nc


def kernel(x, weight):
    x = np.ascontiguousarray(np.asarray(x), dtype=np.float32)
    w = np.ascontiguousarray(np.asarray(weight), dtype=np.float32)
    assert x.shape == (FULL_M, FULL_K) and w.shape == (FULL_N, FULL_K)

    from concourse.bass_utils import run_bass_kernel_spmd

    nc = build_program()
    in_maps = []
    for c in range(N_CORES):
        r, s = divmod(c, GRID_J)
        in_maps.append({
            "xs": x[r * M_SH:(r + 1) * M_SH],
            "ws": w[s * N_SH:(s + 1) * N_SH],
        })
    res = run_bass_kernel_spmd(nc, in_maps, core_ids=list(range(N_CORES))).results
    outp = np.empty((FULL_M, FULL_N), dtype=np.float32)
    for c in range(N_CORES):
        r, s = divmod(c, GRID_J)
        outp[r * M_SH:(r + 1) * M_SH, s * N_SH:(s + 1) * N_SH] = res[c]["out"]
    return outp


# revision 6
# speedup vs baseline: 1.0110x; 1.0110x over previous
"""Binarized linear: out = sign(x+eps) @ sign(w+eps).T on 8 trn2 cores.

Sharding: 4x2 grid. Core c=(r,s): rows x[r*2048:(r+1)*2048], rows w[s*2048:(s+1)*2048].
Each core computes a [2048, 2048] output block; host concatenates. No collectives.

Per-core kernel:
  - binarize x-shard to fp8e4m3 as +/-0.5 (DVE: (x>=0)-0.5), w-shard as +/-1 (ACT Sign)
  - transpose both to [K-on-partition] layout via plain fp8 matmul against identity
    (PE, exact, counts as PE-busy so HAM stays warm)
  - fp8 DoubleRow matmuls accumulate K=256 per instruction into fp32 PSUM
  - out = 2 * psum (exact: sums of +/-0.5*1 are multiples of 0.5; < 2^24)

Schedule: transpose work is chopped into small groups and pumped between DR
matmuls so the PE never idles and the HAM clock gate stays at 8/8.
"""

from collections import deque

import numpy as np

P = 128
GRID_I, GRID_J = 4, 2
N_CORES = 8
FULL_M, FULL_N, FULL_K = 8192, 4096, 4096
M_SH, N_SH = FULL_M // GRID_I, FULL_N // GRID_J  # 2048, 2048

_PROGRAM_CACHE = {}


def build_program(m_sh=M_SH, n_sh=N_SH, k=FULL_K, use_dr=True, warmup=64,
                  interleave=True):
    """Build (and cache) the per-core Bass program. Same SPMD program on all cores."""
    key = (m_sh, n_sh, k, use_dr, warmup, interleave)
    if key in _PROGRAM_CACHE:
        return _PROGRAM_CACHE[key]

    from contextlib import ExitStack

    import concourse.bass as bass
    import concourse.mybir as mybir
    from concourse import bacc, tile
    from concourse.masks import make_identity

    f32 = mybir.dt.float32
    fp8 = mybir.dt.float8e4

    KT = k // P          # number of 128-wide k tiles
    KH = k // 2          # half-row staging width
    IB = m_sh // P       # i blocks (x rows / 128)
    JB = n_sh // P       # j blocks (w rows / 128)
    JBLK = 512           # j chunk width (matmul free dim)
    JC = n_sh // JBLK    # j chunks
    JB_PER_JC = JBLK // P
    KG = 4               # k-tiles per transpose-evict group
    assert KT % KG == 0 and KT % 2 == 0

    nc = bacc.Bacc("TRN2", target_bir_lowering=False, debug=False)
    xs = nc.dram_tensor("xs", [m_sh, k], f32, kind="ExternalInput").ap()
    ws = nc.dram_tensor("ws", [n_sh, k], f32, kind="ExternalInput").ap()
    out = nc.dram_tensor("out", [m_sh, n_sh], f32, kind="ExternalOutput").ap()

    with tile.TileContext(nc) as tc, ExitStack() as ctx:
        const_pool = ctx.enter_context(tc.tile_pool(name="const", bufs=1))
        stage_pool = ctx.enter_context(tc.tile_pool(name="stage", bufs=2))
        b8_pool = ctx.enter_context(tc.tile_pool(name="b8", bufs=2))
        xbt_pool = ctx.enter_context(tc.tile_pool(name="xbt", bufs=1))
        wbt_pool = ctx.enter_context(tc.tile_pool(name="wbt", bufs=1))
        out_pool = ctx.enter_context(tc.tile_pool(name="outp", bufs=4))
        psum_t = ctx.enter_context(tc.tile_pool(name="pst", bufs=3, space="PSUM"))
        psum_mm = ctx.enter_context(tc.tile_pool(name="psmm", bufs=2, space="PSUM"))

        ident = const_pool.tile([P, P], fp8, tag="ident")
        make_identity(nc, ident)
        sign_bias = const_pool.tile([P, 1], f32, tag="sbias")
        nc.any.memset(sign_bias[:], 1e-20)

        # PE warmup: matmuls with no data dependency so the HAM clock-gate
        # opens to 8/8 while the first input DMAs are still in flight.
        if warmup:
            pw = psum_t.tile([P, P], f32, tag="warm", name="warm")
            for _ in range(warmup):
                nc.tensor.matmul(pw[:], lhsT=ident[:], rhs=ident[:],
                                 start=True, stop=True)

        # Resident transposed binarized operands, K on partitions:
        #   xbT[ib][kp, kt, i] = bin(xs[ib*128 + i, kt*128 + kp])   (+/-0.5)
        #   wbT[jc][kp, kt, j] = bin(ws[jc*512 + j, kt*128 + kp])   (+/-1)
        xbT = [
            xbt_pool.tile([P, KT, P], fp8, tag=f"xbt{ib}", name=f"xbt{ib}")
            for ib in range(IB)
        ]
        wbT = [
            wbt_pool.tile([P, KT, JBLK], fp8, tag=f"wbt{jc}", name=f"wbt{jc}")
            for jc in range(JC)
        ]

        def bin_x(b8h, stgh):
            # (x >= 0) -> {1,0}; minus 0.5 -> +/-0.5. Matches sign(x+1e-20) up
            # to the measure-zero region (-1e-20, 0) that f32 randn never hits.
            nc.vector.tensor_scalar(
                b8h, stgh, 0.0, 0.5,
                mybir.AluOpType.is_ge, mybir.AluOpType.subtract,
            )

        def bin_w(b8h, stgh):
            nc.scalar.sign(b8h, stgh, bias=sign_bias[:])  # sign(w+1e-20) -> +/-1

        def load_binarize(src_rows, stg_tag, b8_tag, binarize):
            """Load 128 rows x k f32 (two half DMAs), binarize to fp8."""
            b8 = b8_pool.tile([P, k], fp8, tag=b8_tag, name=b8_tag)
            for h in range(2):
                stg = stage_pool.tile([P, KH], f32, tag=stg_tag, name=stg_tag)
                nc.sync.dma_start(stg[:], src_rows[:, h * KH:(h + 1) * KH])
                binarize(b8[:, h * KH:(h + 1) * KH], stg[:])
            return b8

        pending = deque()  # transpose-group closures (each ~4 PE matmuls)

        def queue_tgroups(b8, dest, dest_col0):
            for kg in range(KT // KG):
                def g(kg=kg, b8=b8, dest=dest, dest_col0=dest_col0):
                    pt = psum_t.tile([P, KG, P], f32, tag="pt", name="pt")
                    for t in range(KG):
                        kt = kg * KG + t
                        nc.tensor.matmul(
                            pt[:, t, :],
                            lhsT=b8[:, kt * P:(kt + 1) * P],
                            rhs=ident[:],
                            start=True, stop=True,
                        )
                    nc.any.tensor_copy(
                        dest[:, kg * KG:(kg + 1) * KG, dest_col0:dest_col0 + P],
                        pt[:],
                    )
                pending.append(g)

        def pump(n):
            for _ in range(n):
                if not pending:
                    return
                pending.popleft()()

        def prep_x(ib):
            b8 = load_binarize(xs[ib * P:(ib + 1) * P, :], "stgx", "xb8", bin_x)
            queue_tgroups(b8, xbT[ib], 0)

        def prep_w(jb):
            jc, sub = divmod(jb, JB_PER_JC)
            b8 = load_binarize(ws[jb * P:(jb + 1) * P, :], "stgw", "wb8", bin_w)
            queue_tgroups(b8, wbT[jc], sub * P)

        def mm_block(ib, jc, pump_between=False):
            ps = psum_mm.tile([P, JBLK], f32, tag="ps", name="ps")
            if use_dr:
                nk = KT // 2
                for kp in range(nk):
                    nc.tensor.matmul(
                        ps[:],
                        lhsT=xbT[ib][:, 2 * kp:2 * kp + 2, :],
                        rhs=wbT[jc][:, 2 * kp:2 * kp + 2, :],
                        start=(kp == 0), stop=(kp == nk - 1),
                        perf_mode=mybir.MatmulPerfMode.DoubleRow,
                    )
                    if pump_between:
                        pump(1)
            else:
                for kt in range(KT):
                    nc.tensor.matmul(
                        ps[:],
                        lhsT=xbT[ib][:, kt, :],
                        rhs=wbT[jc][:, kt, :],
                        start=(kt == 0), stop=(kt == KT - 1),
                    )
                    if pump_between:
                        pump(1)
            ob = out_pool.tile([P, JBLK], f32, tag="ob", name="ob")
            # products are +/-0.5 (x) * +/-1 (w) = +/-0.5 -> scale by 2
            nc.any.tensor_scalar_mul(ob[:], ps[:], 2.0)
            nc.sync.dma_start(
                out[ib * P:(ib + 1) * P, jc * JBLK:(jc + 1) * JBLK], ob[:]
            )

        if interleave:
            # Startup: the first j-chunk of w plus the first x block.
            for jb in range(JB_PER_JC):
                prep_w(jb)
                pump(len(pending))
            prep_x(0)
            pump(len(pending))
            w_next = JB_PER_JC
            # Pass 0 over jc=0, feeding remaining preps between DR matmuls.
            for ib in range(IB):
                if ib + 1 < IB:
                    prep_x(ib + 1)
                if w_next < JB:
                    prep_w(w_next)
                    w_next += 1
                mm_block(ib, 0, pump_between=True)
            while w_next < JB:
                prep_w(w_next)
                w_next += 1
            pump(len(pending))
            for jc in range(1, JC):
                for ib in range(IB):
                    mm_block(ib, jc)
        else:
            for jb in range(JB):
                prep_w(jb)
                pump(len(pending))
            for ib in range(IB):
                prep_x(ib)
                pump(len(pending))
                mm_block(ib, 0)
            for jc in range(1, JC):
                for ib in range(IB):
                    mm_block(ib, jc)

    nc.compile()
    _PROGRAM_CACHE[key] = nc
    return nc


def kernel(x, weight):
    x = np.ascontiguousarray(np.asarray(x), dtype=np.float32)
    w = np.ascontiguousarray(np.asarray(weight), dtype=np.float32)
    assert x.shape == (FULL_M, FULL_K) and w.shape == (FULL_N, FULL_K)

    from concourse.bass_utils import run_bass_kernel_spmd

    nc = build_program()
    in_maps = []
    for c in range(N_CORES):
        r, s = divmod(c, GRID_J)
        in_maps.append({
            "xs": x[r * M_SH:(r + 1) * M_SH],
            "ws": w[s * N_SH:(s + 1) * N_SH],
        })
    res = run_bass_kernel_spmd(nc, in_maps, core_ids=list(range(N_CORES))).results
    outp = np.empty((FULL_M, FULL_N), dtype=np.float32)
    for c in range(N_CORES):
        r, s = divmod(c, GRID_J)
        outp[r * M_SH:(r + 1) * M_SH, s * N_SH:(s + 1) * N_SH] = res[c]["out"]
    return outp


# revision 10
# speedup vs baseline: 1.1229x; 1.1107x over previous
"""Binarized linear: out = sign(x+eps) @ sign(w+eps).T on 8 trn2 cores.

Sharding: 4x2 grid. Core c=(r,s): rows x[r*2048:(r+1)*2048], rows w[s*2048:(s+1)*2048].
Each core computes a [2048, 2048] output block; host concatenates. No collectives.

Per-core kernel:
  - binarize x-shard to fp8e4m3 as +/-0.5 (DVE: (x>=0)-0.5), w-shard as +/-1 (ACT Sign)
  - transpose both to [K-on-partition] layout via plain fp8 matmul against identity
    (PE, exact, counts as PE-busy so HAM stays warm)
  - fp8 DoubleRow matmuls accumulate K=256 per instruction into fp32 PSUM
  - out = 2 * psum (exact: sums of +/-0.5*1 are multiples of 0.5; < 2^24)

Schedule: transpose work is chopped into small groups and pumped between DR
matmuls so the PE never idles and the HAM clock gate stays at 8/8.
"""

from collections import deque

import numpy as np

P = 128
GRID_I, GRID_J = 4, 2
N_CORES = 8
FULL_M, FULL_N, FULL_K = 8192, 4096, 4096
M_SH, N_SH = FULL_M // GRID_I, FULL_N // GRID_J  # 2048, 2048

_PROGRAM_CACHE = {}


def build_program(m_sh=M_SH, n_sh=N_SH, k=FULL_K, use_dr=True, warmup=64,
                  interleave=True):
    """Build (and cache) the per-core Bass program. Same SPMD program on all cores."""
    key = (m_sh, n_sh, k, use_dr, warmup, interleave)
    if key in _PROGRAM_CACHE:
        return _PROGRAM_CACHE[key]

    from contextlib import ExitStack

    import concourse.bass as bass
    import concourse.mybir as mybir
    from concourse import bacc, tile
    from concourse.masks import make_identity

    f32 = mybir.dt.float32
    fp8 = mybir.dt.float8e4

    KT = k // P          # number of 128-wide k tiles
    KH = k // 2          # half-row staging width
    IB = m_sh // P       # i blocks (x rows / 128)
    JB = n_sh // P       # j blocks (w rows / 128)
    JBLK = 512           # j chunk width (matmul free dim)
    JC = n_sh // JBLK    # j chunks
    JB_PER_JC = JBLK // P
    KG = 4               # k-tiles per transpose-evict group
    assert KT % KG == 0 and KT % 2 == 0

    nc = bacc.Bacc("TRN2", target_bir_lowering=False, debug=False)
    xs = nc.dram_tensor("xs", [m_sh, k], f32, kind="ExternalInput").ap()
    ws = nc.dram_tensor("ws", [n_sh, k], f32, kind="ExternalInput").ap()
    out = nc.dram_tensor("out", [m_sh, n_sh], f32, kind="ExternalOutput").ap()

    with tile.TileContext(nc) as tc, ExitStack() as ctx:
        const_pool = ctx.enter_context(tc.tile_pool(name="const", bufs=1))
        stage_pool = ctx.enter_context(tc.tile_pool(name="stage", bufs=2))
        b8_pool = ctx.enter_context(tc.tile_pool(name="b8", bufs=2))
        xbt_pool = ctx.enter_context(tc.tile_pool(name="xbt", bufs=1))
        wbt_pool = ctx.enter_context(tc.tile_pool(name="wbt", bufs=1))
        out_pool = ctx.enter_context(tc.tile_pool(name="outp", bufs=4))
        psum_t = ctx.enter_context(tc.tile_pool(name="pst", bufs=3, space="PSUM"))
        psum_mm = ctx.enter_context(tc.tile_pool(name="psmm", bufs=3, space="PSUM"))

        ident = const_pool.tile([P, P], fp8, tag="ident")
        make_identity(nc, ident)
        sign_bias = const_pool.tile([P, 1], f32, tag="sbias")
        nc.any.memset(sign_bias[:], 1e-20)

        # PE warmup: matmuls with no data dependency so the HAM clock-gate
        # opens to 8/8 while the first input DMAs are still in flight.
        psum_wu = ctx.enter_context(tc.tile_pool(name="pswu", bufs=1, space="PSUM"))
        warm_psum = psum_wu.tile([P, P], f32, tag="warm", name="warm") if warmup else None

        def warm(n):
            for _ in range(n):
                nc.tensor.matmul(warm_psum[:], lhsT=ident[:], rhs=ident[:],
                                 start=True, stop=True)

        if warmup:
            warm(warmup)

        # Resident transposed binarized operands, K on partitions:
        #   xbT[ib][kp, kt, i] = bin(xs[ib*128 + i, kt*128 + kp])   (+/-0.5)
        #   wbT[jc][kp, kt, j] = bin(ws[jc*512 + j, kt*128 + kp])   (+/-1)
        xbT = [
            xbt_pool.tile([P, KT, P], fp8, tag=f"xbt{ib}", name=f"xbt{ib}")
            for ib in range(IB)
        ]
        wbT = [
            wbt_pool.tile([P, KT, JBLK], fp8, tag=f"wbt{jc}", name=f"wbt{jc}")
            for jc in range(JC)
        ]

        def bin_x(b8h, stgh):
            # (x >= 0) -> {1,0}; minus 0.5 -> +/-0.5. Matches sign(x+1e-20) up
            # to the measure-zero region (-1e-20, 0) that f32 randn never hits.
            nc.vector.tensor_scalar(
                b8h, stgh, 0.0, 0.5,
                mybir.AluOpType.is_ge, mybir.AluOpType.subtract,
            )

        def bin_w(b8h, stgh):
            nc.scalar.sign(b8h, stgh, bias=sign_bias[:])  # sign(w+1e-20) -> +/-1

        def load_binarize(src_rows, stg_tag, b8_tag, binarize):
            """Load 128 rows x k f32 (two half DMAs), binarize to fp8."""
            b8 = b8_pool.tile([P, k], fp8, tag=b8_tag, name=b8_tag)
            for h in range(2):
                stg = stage_pool.tile([P, KH], f32, tag=stg_tag, name=stg_tag)
                nc.sync.dma_start(stg[:], src_rows[:, h * KH:(h + 1) * KH])
                binarize(b8[:, h * KH:(h + 1) * KH], stg[:])
            return b8

        pending = deque()  # transpose-group closures (each ~4 PE matmuls)

        def queue_tgroups(b8, dest, dest_col0):
            for kg in range(KT // KG):
                def g(kg=kg, b8=b8, dest=dest, dest_col0=dest_col0):
                    pt = psum_t.tile([P, KG, P], f32, tag="pt", name="pt")
                    for t in range(KG):
                        kt = kg * KG + t
                        nc.tensor.matmul(
                            pt[:, t, :],
                            lhsT=b8[:, kt * P:(kt + 1) * P],
                            rhs=ident[:],
                            start=True, stop=True,
                        )
                    nc.any.tensor_copy(
                        dest[:, kg * KG:(kg + 1) * KG, dest_col0:dest_col0 + P],
                        pt[:],
                    )
                pending.append(g)

        def pump(n):
            for _ in range(n):
                if not pending:
                    return
                pending.popleft()()

        def prep_x(ib):
            b8 = load_binarize(xs[ib * P:(ib + 1) * P, :], "stgx", "xb8", bin_x)
            queue_tgroups(b8, xbT[ib], 0)

        def prep_w(jb):
            jc, sub = divmod(jb, JB_PER_JC)
            b8 = load_binarize(ws[jb * P:(jb + 1) * P, :], "stgw", "wb8", bin_w)
            queue_tgroups(b8, wbT[jc], sub * P)

        def mm_block(ib, jc, pump_between=False):
            ps = psum_mm.tile([P, JBLK], f32, tag="ps", name="ps")
            if use_dr:
                nk = KT // 2
                for kp in range(nk):
                    nc.tensor.matmul(
                        ps[:],
                        lhsT=xbT[ib][:, 2 * kp:2 * kp + 2, :],
                        rhs=wbT[jc][:, 2 * kp:2 * kp + 2, :],
                        start=(kp == 0), stop=(kp == nk - 1),
                        perf_mode=mybir.MatmulPerfMode.DoubleRow,
                    )
                    if pump_between:
                        pump(1)
            else:
                for kt in range(KT):
                    nc.tensor.matmul(
                        ps[:],
                        lhsT=xbT[ib][:, kt, :],
                        rhs=wbT[jc][:, kt, :],
                        start=(kt == 0), stop=(kt == KT - 1),
                    )
                    if pump_between:
                        pump(1)
            ob = out_pool.tile([P, JBLK], f32, tag="ob", name="ob")
            # products are +/-0.5 (x) * +/-1 (w) = +/-0.5 -> scale by 2
            nc.any.tensor_scalar_mul(ob[:], ps[:], 2.0)
            nc.sync.dma_start(
                out[ib * P:(ib + 1) * P, jc * JBLK:(jc + 1) * JBLK], ob[:]
            )

        if interleave:
            # Startup: the first j-chunk of w plus the first x block. Pad the
            # PE stream with warmup matmuls so the HAM window never sees idle
            # while the startup DMAs land.
            for jb in range(JB_PER_JC):
                prep_w(jb)
                while pending:
                    pump(1)
                    if warmup:
                        warm(4)
            prep_x(0)
            while pending:
                pump(1)
                if warmup:
                    warm(4)
            # Pass jc: prep x (pass 0) and the NEXT pass's w chunk between DR
            # matmuls, spreading the input DMA across the whole schedule.
            w_next = JB_PER_JC
            for jc in range(JC):
                for ib in range(IB):
                    if jc == 0 and ib + 1 < IB:
                        prep_x(ib + 1)
                    # 4 w blocks per pass, one every IB//4 iterations
                    if jc < JC - 1 and ib % (IB // JB_PER_JC) == 0 and w_next < JB:
                        prep_w(w_next)
                        w_next += 1
                    mm_block(ib, jc, pump_between=True)
            pump(len(pending))
        else:
            for jb in range(JB):
                prep_w(jb)
                pump(len(pending))
            for ib in range(IB):
                prep_x(ib)
                pump(len(pending))
                mm_block(ib, 0)
            for jc in range(1, JC):
                for ib in range(IB):
                    mm_block(ib, jc)

    nc.compile()
    _PROGRAM_CACHE[key] = nc
    return nc


def kernel(x, weight):
    x = np.ascontiguousarray(np.asarray(x), dtype=np.float32)
    w = np.ascontiguousarray(np.asarray(weight), dtype=np.float32)
    assert x.shape == (FULL_M, FULL_K) and w.shape == (FULL_N, FULL_K)

    from concourse.bass_utils import run_bass_kernel_spmd

    nc = build_program()
    in_maps = []
    for c in range(N_CORES):
        r, s = divmod(c, GRID_J)
        in_maps.append({
            "xs": x[r * M_SH:(r + 1) * M_SH],
            "ws": w[s * N_SH:(s + 1) * N_SH],
        })
    res = run_bass_kernel_spmd(nc, in_maps, core_ids=list(range(N_CORES))).results
    outp = np.empty((FULL_M, FULL_N), dtype=np.float32)
    for c in range(N_CORES):
        r, s = divmod(c, GRID_J)
        outp[r * M_SH:(r + 1) * M_SH, s * N_SH:(s + 1) * N_SH] = res[c]["out"]
    return outp


# revision 11
# speedup vs baseline: 1.1509x; 1.0249x over previous
"""Binarized linear: out = sign(x+eps) @ sign(w+eps).T on 8 trn2 cores.

Sharding: 4x2 grid. Core c=(r,s): rows x[r*2048:(r+1)*2048], rows w[s*2048:(s+1)*2048].
Each core computes a [2048, 2048] output block; host concatenates. No collectives.

Per-core kernel:
  - binarize x-shard to fp8e4m3 as +/-0.5 (DVE: (x>=0)-0.5), w-shard as +/-1 (ACT Sign)
  - transpose both to [K-on-partition] layout via plain fp8 matmul against identity
    (PE, exact, counts as PE-busy so HAM stays warm)
  - fp8 DoubleRow matmuls accumulate K=256 per instruction into fp32 PSUM
  - out = 2 * psum (exact: sums of +/-0.5*1 are multiples of 0.5; < 2^24)

Schedule: transpose work is chopped into small groups and pumped between DR
matmuls so the PE never idles and the HAM clock gate stays at 8/8.
"""

from collections import deque

import numpy as np

P = 128
GRID_I, GRID_J = 4, 2
N_CORES = 8
FULL_M, FULL_N, FULL_K = 8192, 4096, 4096
M_SH, N_SH = FULL_M // GRID_I, FULL_N // GRID_J  # 2048, 2048

_PROGRAM_CACHE = {}


def build_program(m_sh=M_SH, n_sh=N_SH, k=FULL_K, use_dr=True, warmup=64,
                  interleave=True):
    """Build (and cache) the per-core Bass program. Same SPMD program on all cores."""
    key = (m_sh, n_sh, k, use_dr, warmup, interleave)
    if key in _PROGRAM_CACHE:
        return _PROGRAM_CACHE[key]

    from contextlib import ExitStack

    import concourse.bass as bass
    import concourse.mybir as mybir
    from concourse import bacc, tile
    from concourse.masks import make_identity

    f32 = mybir.dt.float32
    fp8 = mybir.dt.float8e4

    KT = k // P          # number of 128-wide k tiles
    KH = k // 2          # half-row staging width
    IB = m_sh // P       # i blocks (x rows / 128)
    JB = n_sh // P       # j blocks (w rows / 128)
    JBLK = 512           # j chunk width (matmul free dim)
    JC = n_sh // JBLK    # j chunks
    JB_PER_JC = JBLK // P
    KG = 4               # k-tiles per transpose-evict group
    assert KT % KG == 0 and KT % 2 == 0

    nc = bacc.Bacc("TRN2", target_bir_lowering=False, debug=False)
    xs = nc.dram_tensor("xs", [m_sh, k], f32, kind="ExternalInput").ap()
    ws = nc.dram_tensor("ws", [n_sh, k], f32, kind="ExternalInput").ap()
    out = nc.dram_tensor("out", [m_sh, n_sh], f32, kind="ExternalOutput").ap()

    with tile.TileContext(nc) as tc, ExitStack() as ctx:
        const_pool = ctx.enter_context(tc.tile_pool(name="const", bufs=1))
        stage_pool = ctx.enter_context(tc.tile_pool(name="stage", bufs=2))
        b8_pool = ctx.enter_context(tc.tile_pool(name="b8", bufs=3))
        xbt_pool = ctx.enter_context(tc.tile_pool(name="xbt", bufs=1))
        wbt_pool = ctx.enter_context(tc.tile_pool(name="wbt", bufs=1))
        out_pool = ctx.enter_context(tc.tile_pool(name="outp", bufs=2))
        psum_t = ctx.enter_context(tc.tile_pool(name="pst", bufs=4, space="PSUM"))
        psum_mm = ctx.enter_context(tc.tile_pool(name="psmm", bufs=3, space="PSUM"))

        ident = const_pool.tile([P, P], fp8, tag="ident")
        make_identity(nc, ident)
        sign_bias = const_pool.tile([P, 1], f32, tag="sbias")
        nc.any.memset(sign_bias[:], 1e-20)

        # PE warmup: matmuls with no data dependency so the HAM clock-gate
        # opens to 8/8 while the first input DMAs are still in flight.
        psum_wu = ctx.enter_context(tc.tile_pool(name="pswu", bufs=1, space="PSUM"))
        warm_psum = psum_wu.tile([P, P], f32, tag="warm", name="warm") if warmup else None

        def warm(n):
            for _ in range(n):
                nc.tensor.matmul(warm_psum[:], lhsT=ident[:], rhs=ident[:],
                                 start=True, stop=True)

        if warmup:
            warm(warmup)

        # Resident transposed binarized operands, K on partitions:
        #   xbT[ib][kp, kt, i] = bin(xs[ib*128 + i, kt*128 + kp])   (+/-0.5)
        #   wbT[jc][kp, kt, j] = bin(ws[jc*512 + j, kt*128 + kp])   (+/-1)
        xbT = [
            xbt_pool.tile([P, KT, P], fp8, tag=f"xbt{ib}", name=f"xbt{ib}")
            for ib in range(IB)
        ]
        wbT = [
            wbt_pool.tile([P, KT, JBLK], fp8, tag=f"wbt{jc}", name=f"wbt{jc}")
            for jc in range(JC)
        ]

        def bin_x(b8h, stgh):
            # (x >= 0) -> {1,0}; minus 0.5 -> +/-0.5. Matches sign(x+1e-20) up
            # to the measure-zero region (-1e-20, 0) that f32 randn never hits.
            nc.vector.tensor_scalar(
                b8h, stgh, 0.0, 0.5,
                mybir.AluOpType.is_ge, mybir.AluOpType.subtract,
            )

        def bin_w(b8h, stgh):
            nc.scalar.sign(b8h, stgh, bias=sign_bias[:])  # sign(w+1e-20) -> +/-1

        def load_binarize(src_rows, stg_tag, b8_tag, binarize):
            """Load 128 rows x k f32 (two half DMAs), binarize to fp8."""
            b8 = b8_pool.tile([P, k], fp8, tag=b8_tag, name=b8_tag)
            for h in range(2):
                stg = stage_pool.tile([P, KH], f32, tag=stg_tag, name=stg_tag)
                nc.sync.dma_start(stg[:], src_rows[:, h * KH:(h + 1) * KH])
                binarize(b8[:, h * KH:(h + 1) * KH], stg[:])
            return b8

        pending = deque()  # transpose-group closures (each ~4 PE matmuls)

        def queue_tgroups(b8, dest, dest_col0):
            for kg in range(KT // KG):
                def g(kg=kg, b8=b8, dest=dest, dest_col0=dest_col0):
                    pt = psum_t.tile([P, KG, P], f32, tag="pt", name="pt")
                    for t in range(KG):
                        kt = kg * KG + t
                        nc.tensor.matmul(
                            pt[:, t, :],
                            lhsT=b8[:, kt * P:(kt + 1) * P],
                            rhs=ident[:],
                            start=True, stop=True,
                        )
                    nc.any.tensor_copy(
                        dest[:, kg * KG:(kg + 1) * KG, dest_col0:dest_col0 + P],
                        pt[:],
                    )
                pending.append(g)

        def pump(n):
            for _ in range(n):
                if not pending:
                    return
                pending.popleft()()

        def prep_x(ib):
            b8 = load_binarize(xs[ib * P:(ib + 1) * P, :], "stgx", "xb8", bin_x)
            queue_tgroups(b8, xbT[ib], 0)

        def prep_w(jb):
            jc, sub = divmod(jb, JB_PER_JC)
            b8 = load_binarize(ws[jb * P:(jb + 1) * P, :], "stgw", "wb8", bin_w)
            queue_tgroups(b8, wbT[jc], sub * P)

        def mm_block(ib, jc, pump_between=False):
            ps = psum_mm.tile([P, JBLK], f32, tag="ps", name="ps")
            if use_dr:
                nk = KT // 2
                for kp in range(nk):
                    nc.tensor.matmul(
                        ps[:],
                        lhsT=xbT[ib][:, 2 * kp:2 * kp + 2, :],
                        rhs=wbT[jc][:, 2 * kp:2 * kp + 2, :],
                        start=(kp == 0), stop=(kp == nk - 1),
                        perf_mode=mybir.MatmulPerfMode.DoubleRow,
                    )
                    if pump_between:
                        pump(1)
            else:
                for kt in range(KT):
                    nc.tensor.matmul(
                        ps[:],
                        lhsT=xbT[ib][:, kt, :],
                        rhs=wbT[jc][:, kt, :],
                        start=(kt == 0), stop=(kt == KT - 1),
                    )
                    if pump_between:
                        pump(1)
            ob = out_pool.tile([P, JBLK], f32, tag="ob", name="ob")
            # products are +/-0.5 (x) * +/-1 (w) = +/-0.5 -> scale by 2
            nc.any.tensor_scalar_mul(ob[:], ps[:], 2.0)
            nc.sync.dma_start(
                out[ib * P:(ib + 1) * P, jc * JBLK:(jc + 1) * JBLK], ob[:]
            )

        if interleave:
            # Startup: the first j-chunk of w plus the first x block. Pad the
            # PE stream with warmup matmuls so the HAM window never sees idle
            # while the startup DMAs land.
            for jb in range(JB_PER_JC):
                prep_w(jb)
                while pending:
                    pump(1)
                    if warmup:
                        warm(4)
            prep_x(0)
            while pending:
                pump(1)
                if warmup:
                    warm(4)
            # Pass jc: prep x (pass 0) and the NEXT pass's w chunk between DR
            # matmuls, spreading the input DMA across the whole schedule.
            w_next = JB_PER_JC
            for jc in range(JC):
                for ib in range(IB):
                    if jc == 0 and ib + 1 < IB:
                        prep_x(ib + 1)
                    # 4 w blocks per pass, one every IB//4 iterations
                    if jc < JC - 1 and ib % (IB // JB_PER_JC) == 0 and w_next < JB:
                        prep_w(w_next)
                        w_next += 1
                    mm_block(ib, jc, pump_between=True)
            pump(len(pending))
        else:
            for jb in range(JB):
                prep_w(jb)
                pump(len(pending))
            for ib in range(IB):
                prep_x(ib)
                pump(len(pending))
                mm_block(ib, 0)
            for jc in range(1, JC):
                for ib in range(IB):
                    mm_block(ib, jc)

    nc.compile()
    _PROGRAM_CACHE[key] = nc
    return nc


def kernel(x, weight):
    x = np.ascontiguousarray(np.asarray(x), dtype=np.float32)
    w = np.ascontiguousarray(np.asarray(weight), dtype=np.float32)
    assert x.shape == (FULL_M, FULL_K) and w.shape == (FULL_N, FULL_K)

    from concourse.bass_utils import run_bass_kernel_spmd

    nc = build_program()
    in_maps = []
    for c in range(N_CORES):
        r, s = divmod(c, GRID_J)
        in_maps.append({
            "xs": x[r * M_SH:(r + 1) * M_SH],
            "ws": w[s * N_SH:(s + 1) * N_SH],
        })
    res = run_bass_kernel_spmd(nc, in_maps, core_ids=list(range(N_CORES))).results
    outp = np.empty((FULL_M, FULL_N), dtype=np.float32)
    for c in range(N_CORES):
        r, s = divmod(c, GRID_J)
        outp[r * M_SH:(r + 1) * M_SH, s * N_SH:(s + 1) * N_SH] = res[c]["out"]
    return outp
